# revision 32
# baseline (speedup 1.0000x reference)
"""ConvGRU 3-node chain (gnn_message_passing) on 8 TRN2 NeuronCores.

Strategy: pure data parallelism - 1 batch item per core, weights replicated,
no collectives. Per-core kernel: channels-on-partitions, zero-padded 66x66
spatial layout in the SBUF free dimension; every 3x3 conv = shifted matmuls
accumulating in PSUM; fp32 PSUM accumulation; bias + sigmoid/tanh fused into
the PSUM->SBUF drains on the scalar engine; GRU elementwise on vector.

Precision split (validated vs the fp32 reference, sim rel ~1.5e-2, gate
2e-2; fp16 is NOT an option - fp16 matmuls measure ~6x slower than bf16 on
real TRN2 despite the cost model claiming parity):
  - The three GATES convs (Wg0/1/2, sigmoid outputs damped by ~1/4
    derivative) run fully in fp8e4m3: 8 of the 9 taps packed in 4
    MatmulPerfMode.DoubleRow two-tap pairs (the k-tile dim of the moving AP
    walks tap pairs with strides 2/66), contracting two taps per pass at
    ~1.13x pass cost, + the 9th tap as a plain fp8 matmul => ~1.75x
    measured per-stream speedup. DoubleRow moving APs must be 3-dim
    [K, [step>=2,2], [1, contiguous span]] (4-dim or stride-1 APs FAULT the
    device), so gates streams use 7-row contiguous spans (462 of a 512 PSUM
    bank, halo cols dropped at drain) instead of strided 8-row chunks.
  - fp8 scaling: moving = value*SX (SX=8), weights = value*SW (SW=512),
    drains apply scale 1/(SX*SW) before bias+sigmoid. S8[n] fp8 copies of
    [h; bu] are produced by the DVE (h, after each upd half) and the
    scalar engine (bu, beside each rider drain). Everything else bf16.

Matmul cost on TRN2 is (moving rows) x pe_cycle x cycles-per-row, so the
structure packs many small convs into few dense streams:
  - node-0's bottom-up path (proj0+integ0) depends only on x: precomputed
    for t<8 in 4 startup streams, 4 timesteps packed per 128 partitions.
  - per step: 3 gates streams (fp8 pairs); cand0+integ1 merged over
    CX0=[r0*h0; bu0; p1] (K=128,N=96); cand1+integ2 merged; cand2;
    proj1+proj2 merged over HH=[h1;h0] (K=128,N=64) for next step's p1/p2.
  - cand2 and proj12 co-stream on disjoint PE column halves.
  - each tap's stationary feeds 4 consecutive matmuls (PSUM bank rotation)
    so LDWEIGHTS stays off the critical path.
  - dead tail work skipped (h0[8], h1[9], and their feeders are unused).
"""
import numpy as np

B, T, CIN, H, W = 8, 8, 3, 64, 64
PROJ, CDIM, HID, NUM_NODE = 32, 32, 64, 3
PROCESS_T = T + NUM_NODE - 1  # 10

PW = W + 2                    # padded width 66
IMG = PW * PW                 # 4356
BASEO = 2                     # image offset in the free dim (guard below)
FREE = 4360                   # free size incl guards at both ends
SWEEP_OFF = BASEO + PW        # row-1 col-0 position (GRU elementwise range)
SWEEP_LEN = H * PW            # 4224
NCH = 8                       # chunks per fp16 conv: 8 rows x 64 cols
RPC = H // NCH                # rows per chunk: 8
TAPS = [di * PW + dj for di in (-1, 0, 1) for dj in (-1, 0, 1)]

# fp8 gates: tap pairs (offsets) for DoubleRow + the fp16 single tap (+1,0)
GPAIRS = [(-PW - 1, -PW + 1), (-PW, 0), (-1, 1), (PW - 1, PW + 1)]
GSINGLE = PW                   # (di,dj) = (+1, 0)
SX, SW = 8.0, 512.0            # moving / weight fp8 scales (pow2)
GSPANS = [(7 * c, 7) for c in range(9)] + [(63, 1)]   # (row0, nrows)

N_CORES = 8
_cache = {}


# ------------------------------------------------------------- host packing
def _f16(x):
    import ml_dtypes
    return np.asarray(x, np.float32).astype(ml_dtypes.bfloat16)


def _f8(x):
    import ml_dtypes
    return np.asarray(x, np.float32).astype(ml_dtypes.float8_e4m3)


def _prep_inputs(inputs):
    inp = {k: np.asarray(v, np.float32) for k, v in inputs.items()}
    w = {}
    xp = np.zeros((B, PROCESS_T, CIN, H, W), np.float32)
    xp[:, :T] = inp["x"]
    xb = _f16(xp)

    def pack(blocks, ncols, rows=128):
        out = np.zeros((rows, 9 * ncols), np.float32)
        for k in range(9):
            di, dj = k // 3, k % 3
            for r0, c0, Wt in blocks:
                O, I = Wt.shape[0], Wt.shape[1]
                out[r0:r0 + I, k * ncols + c0:k * ncols + c0 + O] = \
                    Wt[:, :, di, dj].T
        return _f16(out)

    # gates fp8 pairs: moving S8[n] = [h (0-63); bu (64-95)]; Wg in-ch order
    # [bu; h].  W8[k, pair, j, m] = Wg[m, cin(k), tap(pair,j)] * SW
    def packg(Wg):
        Wr = np.zeros((96, 3, 3, 128), np.float32)     # [k, di, dj, m]
        Wr[0:64] = Wg[:, CDIM:].transpose(1, 2, 3, 0)
        Wr[64:96] = Wg[:, :CDIM].transpose(1, 2, 3, 0)
        w8 = np.zeros((96, 4, 2, 128), np.float32)
        for pi, (ta, tb) in enumerate(GPAIRS):
            for j, t in enumerate((ta, tb)):
                di = round(t / PW)
                dj = t - di * PW
                w8[:, pi, j, :] = Wr[:, di + 1, dj + 1, :] * SW
        # 9th tap (+1, 0) as a plain fp8 matmul (1.0 cyc/row), appended
        ws = Wr[:, 2, 1, :] * SW
        return _f8(np.concatenate([w8.reshape(96, 1024), ws], axis=1))

    for n in range(3):
        w[f"wg8_{n}"] = packg(inp[f"Wg{n}"])
    # cand0+integ1: moving CX0 = [rh0; bu0; p1]; cols 0-63 d0, 64-95 bu1
    w["w4"] = pack([(0, 0, inp["Wc0"][:, CDIM:]), (64, 0, inp["Wc0"][:, :CDIM]),
                    (96, 64, inp["Wint1"])], 96)
    w["w5"] = pack([(0, 0, inp["Wc1"][:, CDIM:]), (64, 0, inp["Wc1"][:, :CDIM]),
                    (96, 64, inp["Wint2"])], 96)
    w["w6"] = pack([(0, 0, inp["Wc2"][:, CDIM:]), (64, 0, inp["Wc2"][:, :CDIM])],
                   64, rows=96)
    # proj12: moving HH = [h1 (0-63); h0 (64-127)]; cols 0-31 p1, 32-63 p2.
    w["w7"] = pack([(0, 32, inp["We21"]), (64, 0, inp["We10"])], 64)
    # proj0 startup: x[t] at partitions 32*(t//4)+3*(t%4); out p0[t] at
    # psum parts 32*(t%4). Only t<8 needed.
    w["wp0"] = pack([(32 * (t // 4) + 3 * (t % 4), 32 * (t % 4), inp["Win0"])
                     for t in range(T)], 128, rows=44)
    # integ0 startup: block-diagonal over 4 packed timesteps
    w["wi0"] = pack([(32 * u, 32 * u, inp["Wint0"]) for u in range(4)], 128)

    bias = np.zeros((128, 14), np.float32)
    for n in range(3):
        bias[:, n] = inp[f"bg{n}"]                    # r at 0-63, z at 64-127
    bias[0:64, 3] = inp["bc0"]; bias[64:96, 3] = inp["bint1"]
    bias[0:64, 4] = inp["bc1"]; bias[64:96, 4] = inp["bint2"]
    bias[0:64, 5] = inp["bc2"]
    bias[64:96, 6] = inp["be10"]; bias[96:128, 6] = inp["be21"]
    bias[:, 7] = np.tile(inp["bin0"], 4)
    bias[:, 8] = np.tile(inp["bint0"], 4)
    bias[64:96, 9] = inp["bint1"] * SX                # fp8 bu1 drain
    bias[64:96, 10] = inp["bint2"] * SX               # fp8 bu2 drain
    bias[:, 11] = np.tile(inp["bint0"], 4) * SX       # fp8 bu0 startup
    bias[96:128, 12] = inp["be10"]                    # p1-proj in cand1
    return xb, w, bias


# ------------------------------------------------------------ kernel build
def build(n_repeat=1):
    import concourse.bass as bass
    import concourse.bacc as bacc
    import concourse.mybir as mybir
    from concourse import tile

    f32, f16, f8 = mybir.dt.float32, mybir.dt.bfloat16, mybir.dt.float8e4
    AF = mybir.ActivationFunctionType
    ALU = mybir.AluOpType
    DRM = mybir.MatmulPerfMode.DoubleRow

    nc = bacc.Bacc(None, target_bir_lowering=False)

    x_ext = nc.declare_dram_parameter("x", [PROCESS_T, CIN, H, W], f16,
                                      isOutput=False)
    wshapes = {"w4": (128, 9 * 96), "w5": (128, 9 * 96), "w6": (96, 9 * 64),
               "w7": (128, 9 * 64), "wp0": (44, 9 * 128), "wi0": (128, 9 * 128)}
    w8shapes = {"wg8_0": (96, 1152), "wg8_1": (96, 1152), "wg8_2": (96, 1152)}
    w_ext = {k: nc.declare_dram_parameter(k, list(s), f16, isOutput=False)
             for k, s in wshapes.items()}
    w8_ext = {k: nc.declare_dram_parameter(k, list(s), f8, isOutput=False)
              for k, s in w8shapes.items()}
    bias_ext = nc.declare_dram_parameter("bias", [128, 14], f32, isOutput=False)
    out_ext = nc.declare_dram_parameter("out", [HID, H, W], f32, isOutput=True)

    with tile.TileContext(nc) as tc:
        with (
            tc.tile_pool(name="pers", bufs=1) as pers,
            tc.tile_pool(name="ps", bufs=1, space=bass.MemorySpace.PSUM) as ps,
        ):
            def ptile(nm, shape, dt):
                return pers.tile(shape, dt, name=nm, tag=nm, uniquify=False)

            # S[n]: h at parts 0-63, bu at 64-95 (fp16; feeds the single tap
            #   + cand r*h + upd); S8[n]: same values * SX in fp8 (gates)
            # CX[n]: r*h at 0-63, bu at 64-95, next-p at 96-127 (cand moving)
            # ZD[n]: dense, parts 0-63: z in [0,4096), d in [4096,8192)
            S = [ptile(f"S{n}", [128, FREE], f16) for n in range(3)]
            S8 = [ptile(f"S8{n}", [96, FREE], f8) for n in range(3)]
            RZ0 = ptile("RZ", [128, FREE], f16)
            CX = [ptile(f"CX{n}", [128, FREE], f16) for n in range(3)]
            ZD = [ptile(f"ZD{n}", [64, 2 * H * W], f16) for n in range(3)]
            HH = ptile("HH", [128, FREE], f16)
            XA = ptile("XA", [128, FREE], f16)
            PA = ptile("PA", [128, 2 * FREE], f16)   # p0[t], 4 steps/img
            BA = ptile("BA", [128, 2 * FREE], f16)   # bu0[t]
            BA8 = ptile("BA8", [128, 2 * FREE], f8)  # bu0[t] * SX
            OUTF = ptile("OUTF", [128, H * W // 2], f32)
            WT = {k: ptile(f"w_{k}", [128 if wshapes[k][0] > 96 else 96,
                                      wshapes[k][1]], f16) for k in wshapes}
            W8 = {k: ptile(f"w_{k}", [96, 1152], f8) for k in w8shapes}
            BIAS = ptile("BIAS", [128, 14], f32)

            for k in wshapes:
                nc.sync.dma_start(WT[k][0:wshapes[k][0], :], w_ext[k][:])
            for k in w8shapes:
                nc.sync.dma_start(W8[k][:], w8_ext[k][:])
            nc.sync.dma_start(BIAS[:], bias_ext[:])
            for tns in S + S8 + CX + ZD + [HH, XA, PA, BA, BA8]:
                nc.gpsimd.memset(tns[:], 0.0)
            # gates r/z buffers: node 0 reuses XA (dead after the startup
            # streams) so the hoisted G0(t+1) never WARs against G2(t)'s
            # z-DMA on a shared buffer; nodes 1/2 share RZ0 (a full cand
            # stream separates their uses)
            RZ = [XA, RZ0, RZ0]

            def img3(tns, p0, p1, img=0):
                o = img * FREE + BASEO
                return tns[p0:p1, o:o + IMG].rearrange(
                    "p (r s) -> p r s", r=PW, s=PW)


            def mov(tns, p0, p1, c, d, img=0):
                s = img * FREE + BASEO + (1 + RPC * c) * PW + 1 + d
                return tns[p0:p1, s:s + RPC * PW].rearrange(
                    "p (r s) -> p r s", r=RPC, s=PW)[:, :, 0:W]

            def dst(tns, p0, p1, c, img=0):
                return img3(tns, p0, p1, img)[:, 1 + RPC * c:1 + RPC * (c + 1),
                                              1:1 + W]

            def dstr(tns, p0, p1, r0, nr):
                return img3(tns, p0, p1)[:, 1 + r0:1 + r0 + nr, 1:1 + W]

            def q3(q, p0, p1):
                return q[p0:p1, 0:512].rearrange("p (r s) -> p r s", r=RPC, s=W)

            def qspan(q, p0, p1, nr):
                return q[p0:p1, 0:nr * PW].rearrange(
                    "p (r s) -> p r s", r=nr, s=PW)[:, :, 0:W]

            qn = [0]

            def qtile(tag):
                qn[0] += 1
                return ps.tile([128, 512], f32, name=f"q{qn[0]}", tag=tag,
                               uniquify=True)

            sw = slice(SWEEP_OFF, SWEEP_OFF + SWEEP_LEN)

            def swi(img):
                o = img * FREE + SWEEP_OFF
                return slice(o, o + SWEEP_LEN)

            def zv(n, r0=0, r1=H):
                return ZD[n][0:64, r0 * W:r1 * W].rearrange(
                    "p (r s) -> p r s", r=r1 - r0, s=W)

            def dv(n, r0=0, r1=H):
                return ZD[n][0:64, 4096 + r0 * W:4096 + r1 * W].rearrange(
                    "p (r s) -> p r s", r=r1 - r0, s=W)

            def interior(tns, p0, p1, r0=0, r1=H):
                return img3(tns, p0, p1)[:, 1 + r0:1 + r1, 1:1 + W]

            def rsw(r0, r1):
                o = BASEO + (1 + r0) * PW
                return slice(o, o + (r1 - r0) * PW)

            def chunk_quads(tags, body, drain):
                for g in range(2):
                    qs = [qtile(t) for t in tags]
                    for k in range(9):
                        for i in range(4):
                            body(qs[i], 4 * g + i, k)
                    for i in range(4):
                        drain(qs[i], 4 * g + i)

            QG = ("qg0", "qg1", "qg2", "qg3")
            QC = ("qc0", "qc1", "qc2", "qc3")

            # ---------- gates stream: fp8 DoubleRow span conv
            def gates_stream(n):
                Wt8 = W8[f"wg8_{n}"]

                def body(q, ci, k):
                    r0, nr = GSPANS[ci]
                    span = nr * PW
                    base = BASEO + (1 + r0) * PW + 1
                    if k < 4:
                        ta, tb = GPAIRS[k]
                        nc.tensor.matmul(
                            q[0:128, 0:span],
                            Wt8[0:96, 256 * k:256 * k + 256].rearrange(
                                "k (j m) -> k j m", j=2),
                            bass.AP(S8[n].tensor, base + ta,
                                    [[FREE, 96], [tb - ta, 2], [1, span]]),
                            start=(k == 0), stop=False, perf_mode=DRM)
                    else:
                        nc.tensor.matmul(
                            q[0:128, 0:span],
                            Wt8[0:96, 1024:1152],
                            bass.AP(S8[n].tensor, base + GSINGLE,
                                    [[FREE, 96], [1, span]]),
                            start=False, stop=True)

                def drain(q, ci):
                    # single 128-partition sigmoid drain: r -> RZ[0:64],
                    # z -> RZ[64:128] (one Act op instead of two); r*h on DVE
                    r0, nr = GSPANS[ci]
                    nc.scalar.activation(dstr(RZ[n], 0, 128, r0, nr),
                                         qspan(q, 0, 128, nr),
                                         AF.Sigmoid, bias=BIAS[0:128, n:n + 1],
                                         scale=1.0 / (SX * SW))
                    nc.vector.tensor_tensor(dstr(CX[n], 0, 64, r0, nr),
                                            dstr(RZ[n], 0, 64, r0, nr),
                                            dstr(S[n], 0, 64, r0, nr),
                                            ALU.mult)

                # 10 span chunks in groups of 4/4/2, stationaries reused
                # across the chunks of each group
                for g, gn in ((0, 4), (4, 4), (8, 2)):
                    qs = [qtile(QG[i]) for i in range(gn)]
                    for k in range(5):
                        for i in range(gn):
                            body(qs[i], g + i, k)
                    for i in range(gn):
                        drain(qs[i], g + i)
                # z to the dense layout upd expects (partition shift 64->0):
                # idle DMA engines, consumed ~15us later by upd
                nc.sync.dma_start(zv(n, 0, H),
                                  interior(RZ[n], 64, 128, 0, H))

            def cand_stream(n, rider):
                Wt = WT[("w4", "w5", "w6")[n]]
                K = 128 if rider else 96
                N = 96 if rider else 64
                ncols = 96 if n < 2 else 64

                def body(q, c, k):
                    nc.tensor.matmul(q[0:N, 0:512],
                                     Wt[0:K, k * ncols:k * ncols + N],
                                     mov(CX[n], 0, K, c, TAPS[k]),
                                     start=(k == 0), stop=(k == 8))

                def drain(q, c):
                    nc.scalar.activation(dv(n, RPC * c, RPC * c + RPC),
                                         q3(q, 0, 64),
                                         AF.Tanh, bias=BIAS[0:64, 3 + n:4 + n])
                    if rider:
                        # bu -> CX (cand moving) and S8 (gates moving), both
                        # on DVE; S[.][64:96] has no remaining reader
                        nc.vector.tensor_scalar_add(
                            dst(CX[n + 1], 64, 96, c), q3(q, 64, 96),
                            BIAS[64:96, 3 + n:4 + n])
                        nc.vector.tensor_scalar(
                            dst(S8[n + 1], 64, 96, c), q3(q, 64, 96),
                            SX, BIAS[64:96, 9 + n:10 + n],
                            ALU.mult, ALU.add)

                chunk_quads(QC, body, drain)

            def rider_stream(n):
                Wt = WT[("w4", "w5")[n]]

                def body(q, c, k):
                    nc.tensor.matmul(q[64:96, 0:512],
                                     Wt[96:128, k * 96 + 64:k * 96 + 96],
                                     mov(CX[n], 96, 128, c, TAPS[k]),
                                     start=(k == 0), stop=(k == 8),
                                     tile_position=(96, 64))

                def drain(q, c):
                    nc.vector.tensor_scalar_add(
                        dst(CX[n + 1], 64, 96, c), q3(q, 64, 96),
                        BIAS[64:96, 3 + n:4 + n])
                    nc.vector.tensor_scalar(
                        dst(S8[n + 1], 64, 96, c), q3(q, 64, 96),
                        SX, BIAS[64:96, 9 + n:10 + n],
                        ALU.mult, ALU.add)

                chunk_quads(QC, body, drain)

            def p12_body(q, c, k):
                nc.tensor.matmul(q[64:128, 0:512],
                                 WT["w7"][0:128, k * 64:k * 64 + 64],
                                 mov(HH, 0, 128, c, TAPS[k]),
                                 start=(k == 0), stop=(k == 8),
                                 tile_position=(0, 64))

            def p12_drain(q, c):
                nc.scalar.activation(dst(CX[0], 96, 128, c), q3(q, 64, 96),
                                     AF.Identity, bias=BIAS[64:96, 6:7])
                nc.scalar.activation(dst(CX[1], 96, 128, c), q3(q, 96, 128),
                                     AF.Identity, bias=BIAS[96:128, 6:7])

            def proj12_stream():
                chunk_quads(QG, p12_body, p12_drain)

            def cand2_proj12_fused():
                Wt = WT["w6"]
                for g in range(2):
                    q6 = [qtile(t) for t in QC]
                    q7 = [qtile(t) for t in QG]
                    for k in range(9):
                        for i in range(4):
                            c = 4 * g + i
                            nc.tensor.matmul(q6[i][0:64, 0:512],
                                             Wt[0:96, k * 64:k * 64 + 64],
                                             mov(CX[2], 0, 96, c, TAPS[k]),
                                             start=(k == 0), stop=(k == 8))
                            p12_body(q7[i], c, k)
                    for i in range(4):
                        c = 4 * g + i
                        nc.scalar.activation(dv(2, RPC * c, RPC * c + RPC),
                                             q3(q6[i], 0, 64), AF.Tanh,
                                             bias=BIAS[0:64, 5:6])
                        p12_drain(q7[i], c)

            def proj0_stream(g):
                nt = 2 if g == 2 else 4
                pb, K, N = 32 * g, 3 * nt, 32 * nt

                def body(q, c, k):
                    nc.tensor.matmul(q[0:N, 0:512],
                                     WT["wp0"][pb:pb + K, k * 128:k * 128 + N],
                                     mov(XA, pb, pb + K, c, TAPS[k]),
                                     start=(k == 0), stop=(k == 8))

                def drain(q, c):
                    nc.scalar.activation(dst(PA, 0, N, c, img=g), q3(q, 0, N),
                                         AF.Identity, bias=BIAS[0:N, 7:8])

                chunk_quads(QG, body, drain)

            def integ0_stream(g):
                nt = 2 if g == 2 else 4
                K = N = 32 * nt

                def body(q, c, k):
                    nc.tensor.matmul(q[0:N, 0:512],
                                     WT["wi0"][0:K, k * 128:k * 128 + N],
                                     mov(PA, 0, K, c, TAPS[k], img=g),
                                     start=(k == 0), stop=(k == 8))

                def drain(q, c):
                    nc.scalar.activation(dst(BA, 0, N, c, img=g), q3(q, 0, N),
                                         AF.Identity, bias=BIAS[0:N, 8:9])
                    nc.scalar.activation(dst(BA8, 0, N, c, img=g), q3(q, 0, N),
                                         AF.Identity,
                                         bias=BIAS[0:N, 11:12], scale=SX)

                chunk_quads(QC, body, drain)

            def copy_bu0(t):
                g, u = t // 4, t % 4
                nc.sync.dma_start(CX[0][64:96, sw],
                                  BA[32 * u:32 * u + 32, swi(g)])
                nc.sync.dma_start(S8[0][64:96, sw],
                                  BA8[32 * u:32 * u + 32, swi(g)])

            def upd(n, eng=None, r0=0, r1=H):
                # h' = d + z*(h - d); CX[n] interior used as scratch.
                eng = eng or nc.vector
                ci = interior(CX[n], 0, 64, r0, r1)
                si = interior(S[n], 0, 64, r0, r1)
                eng.tensor_tensor(ci, si, dv(n, r0, r1), ALU.subtract)
                eng.tensor_tensor(ci, zv(n, r0, r1), ci, ALU.mult)
                eng.tensor_tensor(si, dv(n, r0, r1), ci, ALU.add)

            def s8h(n, r0=0, r1=H):
                # refresh the fp8 gates moving copy S8[n] = h * SX (DVE)
                nc.vector.tensor_scalar_mul(
                    interior(S8[n], 0, 64, r0, r1),
                    interior(S[n], 0, 64, r0, r1), SX)

            # ---------- program
            for rep in range(n_repeat):
                # x into XA each rep: XA doubles as gates0's r/z buffer, so
                # its startup contents must be restored for rep > 1
                nc.gpsimd.memset(XA[:], 0.0)
                for t in range(T):
                    pb = 32 * (t // 4) + 3 * (t % 4)
                    nc.sync.dma_start(
                        img3(XA, pb, pb + 3)[:, 1:1 + H, 1:1 + W], x_ext[t])
                for n in range(3):
                    nc.gpsimd.memset(S[n][0:64, :], 0.0)
                    nc.gpsimd.memset(S8[n][0:64, :], 0.0)
                for g in range(2):
                    proj0_stream(g)
                for g in range(2):
                    integ0_stream(g)

                for t in range(PROCESS_T):
                    if t == 0:
                        copy_bu0(0)
                        gates_stream(0)                      # G0(0)
                    if t <= 7:
                        cand_stream(0, rider=(t >= 1))       # C0(t)
                    elif t == 8:
                        rider_stream(0)                      # bu1[8] only
                    if t <= 6:
                        copy_bu0(t + 1)      # early: hoisted G0(t+1) needs it
                    if t <= 7:
                        for hf in (0, 1):                    # upd0 + h0->HH
                            upd(0, r0=32 * hf, r1=32 * hf + 32)
                            nc.sync.dma_start(
                                HH[64:128, rsw(32 * hf, 32 * hf + 32)],
                                S[0][0:64, rsw(32 * hf, 32 * hf + 32)])
                            if t <= 6:       # h0[7] never feeds gates0 again
                                s8h(0, 32 * hf, 32 * hf + 32)
                    if t == 0:
                        nc.sync.dma_start(HH[0:64, sw], S[1][0:64, sw])
                    if 1 <= t <= 8:
                        gates_stream(1)                      # G1(t)
                        cand_stream(1, rider=True)           # C1(t)
                    elif t == 9:
                        rider_stream(1)                      # bu2[9]
                    if 1 <= t <= 8:
                        for hf in (0, 1):                    # upd1 + h1->HH
                            upd(1, r0=32 * hf, r1=32 * hf + 32)
                            nc.sync.dma_start(
                                HH[0:64, rsw(32 * hf, 32 * hf + 32)],
                                S[1][0:64, rsw(32 * hf, 32 * hf + 32)])
                            if t <= 7:       # h1[8] never feeds gates1 again
                                s8h(1, 32 * hf, 32 * hf + 32)
                    if t >= 2:
                        gates_stream(2)                      # G2(t)
                    if t <= 6:
                        gates_stream(0)      # G0(t+1) hoisted: fills the PE
                                             # while cand2's inputs settle
                    if 2 <= t <= 8:
                        cand2_proj12_fused()                 # C2(t) || p12
                    elif t == 9:
                        cand_stream(2, rider=False)          # C2(9)
                    elif t <= 1:
                        proj12_stream()      # p1/p2 startup (t+1)
                    if t >= 2:
                        # t=8 is the tail-critical step: gates2(9) waits on
                        # h2[8]; Pool's 0.42-efficiency chain would stall PE
                        upd(2, nc.vector if t == 8 else nc.gpsimd)
                        if t <= 8:           # h2[9] is the output only
                            s8h(2)

                # output h2 (f32)
                hv = img3(S[2], 0, 64)
                nc.vector.tensor_copy(
                    OUTF[0:64, :].rearrange("p (r s) -> p r s", r=H // 2, s=W),
                    hv[:, 1:1 + H // 2, 1:1 + W])
                nc.scalar.activation(
                    OUTF[64:128, :].rearrange("p (r s) -> p r s", r=H // 2,
                                              s=W),
                    hv[:, 1 + H // 2:1 + H, 1:1 + W], AF.Identity)
                nc.sync.dma_start(
                    out_ext[:, 0:H // 2, :],
                    OUTF[0:64, :].rearrange("p (r s) -> p r s", r=H // 2, s=W))
                nc.sync.dma_start(
                    out_ext[:, H // 2:H, :],
                    OUTF[64:128, :].rearrange("p (r s) -> p r s", r=H // 2,
                                              s=W))

    nc.compile()
    return nc


# ----------------------------------------------------------------- entry
def kernel(**inputs) -> np.ndarray:
    from concourse.bass_utils import run_bass_kernel_spmd
    xb, w, bias = _prep_inputs(inputs)
    if "nc" not in _cache:
        _cache["nc"] = build(1)
    nc = _cache["nc"]
    in_maps = []
    for b in range(N_CORES):
        m = {"x": np.ascontiguousarray(xb[b]), "bias": bias}
        m.update(w)
        in_maps.append(m)
    core_ids = list(range(N_CORES))
    try:
        res = run_bass_kernel_spmd(nc, in_maps, core_ids=core_ids).results
    except Exception:
        # transient device wedge has been observed; a clean retry recovers it
        res = run_bass_kernel_spmd(nc, in_maps, core_ids=core_ids).results
    return np.stack([res[b]["out"] for b in range(N_CORES)]).astype(np.float32)


# revision 33
# speedup vs baseline: 1.0019x; 1.0019x over previous
"""ConvGRU 3-node chain (gnn_message_passing) on 8 TRN2 NeuronCores.

Strategy: pure data parallelism - 1 batch item per core, weights replicated,
no collectives. Per-core kernel: channels-on-partitions, zero-padded 66x66
spatial layout in the SBUF free dimension; every 3x3 conv = shifted matmuls
accumulating in PSUM; fp32 PSUM accumulation; bias + sigmoid/tanh fused into
the PSUM->SBUF drains on the scalar engine; GRU elementwise on vector.

Precision split (validated vs the fp32 reference, sim rel ~1.5e-2, gate
2e-2; fp16 is NOT an option - fp16 matmuls measure ~6x slower than bf16 on
real TRN2 despite the cost model claiming parity):
  - The three GATES convs (Wg0/1/2, sigmoid outputs damped by ~1/4
    derivative) run fully in fp8e4m3: 8 of the 9 taps packed in 4
    MatmulPerfMode.DoubleRow two-tap pairs (the k-tile dim of the moving AP
    walks tap pairs with strides 2/66), contracting two taps per pass at
    ~1.13x pass cost, + the 9th tap as a plain fp8 matmul => ~1.75x
    measured per-stream speedup. DoubleRow moving APs must be 3-dim
    [K, [step>=2,2], [1, contiguous span]] (4-dim or stride-1 APs FAULT the
    device), so gates streams use 7-row contiguous spans (462 of a 512 PSUM
    bank, halo cols dropped at drain) instead of strided 8-row chunks.
  - fp8 scaling: moving = value*SX (SX=8), weights = value*SW (SW=512),
    drains apply scale 1/(SX*SW) before bias+sigmoid. S8[n] fp8 copies of
    [h; bu] are produced by the DVE (h, after each upd half) and the
    scalar engine (bu, beside each rider drain). Everything else bf16.

Matmul cost on TRN2 is (moving rows) x pe_cycle x cycles-per-row, so the
structure packs many small convs into few dense streams:
  - node-0's bottom-up path (proj0+integ0) depends only on x: precomputed
    for t<8 in 4 startup streams, 4 timesteps packed per 128 partitions.
  - per step: 3 gates streams (fp8 pairs); cand0+integ1 merged over
    CX0=[r0*h0; bu0; p1] (K=128,N=96); cand1+integ2 merged; cand2;
    proj1+proj2 merged over HH=[h1;h0] (K=128,N=64) for next step's p1/p2.
  - cand2 and proj12 co-stream on disjoint PE column halves.
  - each tap's stationary feeds 4 consecutive matmuls (PSUM bank rotation)
    so LDWEIGHTS stays off the critical path.
  - dead tail work skipped (h0[8], h1[9], and their feeders are unused).
"""
import numpy as np

B, T, CIN, H, W = 8, 8, 3, 64, 64
PROJ, CDIM, HID, NUM_NODE = 32, 32, 64, 3
PROCESS_T = T + NUM_NODE - 1  # 10

PW = W + 2                    # padded width 66
IMG = PW * PW                 # 4356
BASEO = 2                     # image offset in the free dim (guard below)
FREE = 4360                   # free size incl guards at both ends
SWEEP_OFF = BASEO + PW        # row-1 col-0 position (GRU elementwise range)
SWEEP_LEN = H * PW            # 4224
NCH = 8                       # chunks per fp16 conv: 8 rows x 64 cols
RPC = H // NCH                # rows per chunk: 8
TAPS = [di * PW + dj for di in (-1, 0, 1) for dj in (-1, 0, 1)]

# fp8 gates: tap pairs (offsets) for DoubleRow + the fp16 single tap (+1,0)
GPAIRS = [(-PW - 1, -PW + 1), (-PW, 0), (-1, 1), (PW - 1, PW + 1)]
GSINGLE = PW                   # (di,dj) = (+1, 0)
SX, SW = 8.0, 512.0            # moving / weight fp8 scales (pow2)
GSPANS = [(7 * c, 7) for c in range(9)] + [(63, 1)]   # (row0, nrows)

N_CORES = 8
_cache = {}


# ------------------------------------------------------------- host packing
def _f16(x):
    import ml_dtypes
    return np.asarray(x, np.float32).astype(ml_dtypes.bfloat16)


def _f8(x):
    import ml_dtypes
    return np.asarray(x, np.float32).astype(ml_dtypes.float8_e4m3)


def _prep_inputs(inputs):
    inp = {k: np.asarray(v, np.float32) for k, v in inputs.items()}
    w = {}
    xp = np.zeros((B, PROCESS_T, CIN, H, W), np.float32)
    xp[:, :T] = inp["x"]
    xb = _f16(xp)

    def pack(blocks, ncols, rows=128):
        out = np.zeros((rows, 9 * ncols), np.float32)
        for k in range(9):
            di, dj = k // 3, k % 3
            for r0, c0, Wt in blocks:
                O, I = Wt.shape[0], Wt.shape[1]
                out[r0:r0 + I, k * ncols + c0:k * ncols + c0 + O] = \
                    Wt[:, :, di, dj].T
        return _f16(out)

    # gates fp8 pairs: moving S8[n] = [h (0-63); bu (64-95)]; Wg in-ch order
    # [bu; h].  W8[k, pair, j, m] = Wg[m, cin(k), tap(pair,j)] * SW
    def packg(Wg):
        Wr = np.zeros((96, 3, 3, 128), np.float32)     # [k, di, dj, m]
        Wr[0:64] = Wg[:, CDIM:].transpose(1, 2, 3, 0)
        Wr[64:96] = Wg[:, :CDIM].transpose(1, 2, 3, 0)
        w8 = np.zeros((96, 4, 2, 128), np.float32)
        for pi, (ta, tb) in enumerate(GPAIRS):
            for j, t in enumerate((ta, tb)):
                di = round(t / PW)
                dj = t - di * PW
                w8[:, pi, j, :] = Wr[:, di + 1, dj + 1, :] * SW
        # 9th tap (+1, 0) as a plain fp8 matmul (1.0 cyc/row), appended
        ws = Wr[:, 2, 1, :] * SW
        return _f8(np.concatenate([w8.reshape(96, 1024), ws], axis=1))

    for n in range(3):
        w[f"wg8_{n}"] = packg(inp[f"Wg{n}"])
    # cand0+integ1: moving CX0 = [rh0; bu0; p1]; cols 0-63 d0, 64-95 bu1
    w["w4"] = pack([(0, 0, inp["Wc0"][:, CDIM:]), (64, 0, inp["Wc0"][:, :CDIM]),
                    (96, 64, inp["Wint1"])], 96)
    w["w5"] = pack([(0, 0, inp["Wc1"][:, CDIM:]), (64, 0, inp["Wc1"][:, :CDIM]),
                    (96, 64, inp["Wint2"])], 96)
    w["w6"] = pack([(0, 0, inp["Wc2"][:, CDIM:]), (64, 0, inp["Wc2"][:, :CDIM])],
                   64, rows=96)
    # proj12: moving HH = [h1 (0-63); h0 (64-127)]; cols 0-31 p1, 32-63 p2.
    w["w7"] = pack([(0, 32, inp["We21"]), (64, 0, inp["We10"])], 64)
    # proj0 startup: x[t] at partitions 32*(t//4)+3*(t%4); out p0[t] at
    # psum parts 32*(t%4). Only t<8 needed.
    w["wp0"] = pack([(32 * (t // 4) + 3 * (t % 4), 32 * (t % 4), inp["Win0"])
                     for t in range(T)], 128, rows=44)
    # integ0 startup: block-diagonal over 4 packed timesteps
    w["wi0"] = pack([(32 * u, 32 * u, inp["Wint0"]) for u in range(4)], 128)

    bias = np.zeros((128, 14), np.float32)
    for n in range(3):
        bias[:, n] = inp[f"bg{n}"]                    # r at 0-63, z at 64-127
    bias[0:64, 3] = inp["bc0"]; bias[64:96, 3] = inp["bint1"]
    bias[0:64, 4] = inp["bc1"]; bias[64:96, 4] = inp["bint2"]
    bias[0:64, 5] = inp["bc2"]
    bias[64:96, 6] = inp["be10"]; bias[96:128, 6] = inp["be21"]
    bias[:, 7] = np.tile(inp["bin0"], 4)
    bias[:, 8] = np.tile(inp["bint0"], 4)
    bias[64:96, 9] = inp["bint1"] * SX                # fp8 bu1 drain
    bias[64:96, 10] = inp["bint2"] * SX               # fp8 bu2 drain
    bias[:, 11] = np.tile(inp["bint0"], 4) * SX       # fp8 bu0 startup
    bias[96:128, 12] = inp["be10"]                    # p1-proj in cand1
    return xb, w, bias


# ------------------------------------------------------------ kernel build
def build(n_repeat=1):
    import concourse.bass as bass
    import concourse.bacc as bacc
    import concourse.mybir as mybir
    from concourse import tile

    f32, f16, f8 = mybir.dt.float32, mybir.dt.bfloat16, mybir.dt.float8e4
    AF = mybir.ActivationFunctionType
    ALU = mybir.AluOpType
    DRM = mybir.MatmulPerfMode.DoubleRow

    nc = bacc.Bacc(None, target_bir_lowering=False)

    x_ext = nc.declare_dram_parameter("x", [PROCESS_T, CIN, H, W], f16,
                                      isOutput=False)
    wshapes = {"w4": (128, 9 * 96), "w5": (128, 9 * 96), "w6": (96, 9 * 64),
               "w7": (128, 9 * 64), "wp0": (44, 9 * 128), "wi0": (128, 9 * 128)}
    w8shapes = {"wg8_0": (96, 1152), "wg8_1": (96, 1152), "wg8_2": (96, 1152)}
    w_ext = {k: nc.declare_dram_parameter(k, list(s), f16, isOutput=False)
             for k, s in wshapes.items()}
    w8_ext = {k: nc.declare_dram_parameter(k, list(s), f8, isOutput=False)
              for k, s in w8shapes.items()}
    bias_ext = nc.declare_dram_parameter("bias", [128, 14], f32, isOutput=False)
    out_ext = nc.declare_dram_parameter("out", [HID, H, W], f32, isOutput=True)

    with tile.TileContext(nc) as tc:
        with (
            tc.tile_pool(name="pers", bufs=1) as pers,
            tc.tile_pool(name="ps", bufs=1, space=bass.MemorySpace.PSUM) as ps,
        ):
            def ptile(nm, shape, dt):
                return pers.tile(shape, dt, name=nm, tag=nm, uniquify=False)

            # S[n]: h at parts 0-63, bu at 64-95 (fp16; feeds the single tap
            #   + cand r*h + upd); S8[n]: same values * SX in fp8 (gates)
            # CX[n]: r*h at 0-63, bu at 64-95, next-p at 96-127 (cand moving)
            # ZD[n]: dense, parts 0-63: z in [0,4096), d in [4096,8192)
            S = [ptile(f"S{n}", [128, FREE], f16) for n in range(3)]
            S8 = [ptile(f"S8{n}", [96, FREE], f8) for n in range(3)]
            RZ0 = ptile("RZ", [128, FREE], f16)
            CX = [ptile(f"CX{n}", [128, FREE], f16) for n in range(3)]
            ZD = [ptile(f"ZD{n}", [64, 2 * H * W], f16) for n in range(3)]
            HH = ptile("HH", [128, FREE], f16)
            XA = ptile("XA", [128, FREE], f16)
            PA = ptile("PA", [128, 2 * FREE], f16)   # p0[t], 4 steps/img
            BA = ptile("BA", [128, 2 * FREE], f16)   # bu0[t]
            BA8 = ptile("BA8", [128, 2 * FREE], f8)  # bu0[t] * SX
            OUTF = ptile("OUTF", [128, H * W // 2], f32)
            WT = {k: ptile(f"w_{k}", [128 if wshapes[k][0] > 96 else 96,
                                      wshapes[k][1]], f16) for k in wshapes}
            W8 = {k: ptile(f"w_{k}", [96, 1152], f8) for k in w8shapes}
            BIAS = ptile("BIAS", [128, 14], f32)

            for k in wshapes:
                nc.sync.dma_start(WT[k][0:wshapes[k][0], :], w_ext[k][:])
            for k in w8shapes:
                nc.sync.dma_start(W8[k][:], w8_ext[k][:])
            nc.sync.dma_start(BIAS[:], bias_ext[:])
            for tns in S + S8 + CX + ZD + [HH, XA, PA, BA, BA8]:
                nc.gpsimd.memset(tns[:], 0.0)
            # gates r/z buffers: node 0 reuses PA's first image (PA is
            # fully rewritten by proj0's drains each rep and dead after the
            # startup streams) so the hoisted G0(t+1) never WARs against
            # G2(t)'s z-DMA on a shared buffer; nodes 1/2 share RZ0 (a full
            # cand stream separates their uses)
            RZ = [PA, RZ0, RZ0]

            def img3(tns, p0, p1, img=0):
                o = img * FREE + BASEO
                return tns[p0:p1, o:o + IMG].rearrange(
                    "p (r s) -> p r s", r=PW, s=PW)


            def mov(tns, p0, p1, c, d, img=0):
                s = img * FREE + BASEO + (1 + RPC * c) * PW + 1 + d
                return tns[p0:p1, s:s + RPC * PW].rearrange(
                    "p (r s) -> p r s", r=RPC, s=PW)[:, :, 0:W]

            def dst(tns, p0, p1, c, img=0):
                return img3(tns, p0, p1, img)[:, 1 + RPC * c:1 + RPC * (c + 1),
                                              1:1 + W]

            def dstr(tns, p0, p1, r0, nr):
                return img3(tns, p0, p1)[:, 1 + r0:1 + r0 + nr, 1:1 + W]

            def q3(q, p0, p1):
                return q[p0:p1, 0:512].rearrange("p (r s) -> p r s", r=RPC, s=W)

            def qspan(q, p0, p1, nr):
                return q[p0:p1, 0:nr * PW].rearrange(
                    "p (r s) -> p r s", r=nr, s=PW)[:, :, 0:W]

            qn = [0]

            def qtile(tag):
                qn[0] += 1
                return ps.tile([128, 512], f32, name=f"q{qn[0]}", tag=tag,
                               uniquify=True)

            sw = slice(SWEEP_OFF, SWEEP_OFF + SWEEP_LEN)

            def swi(img):
                o = img * FREE + SWEEP_OFF
                return slice(o, o + SWEEP_LEN)

            def zv(n, r0=0, r1=H):
                return ZD[n][0:64, r0 * W:r1 * W].rearrange(
                    "p (r s) -> p r s", r=r1 - r0, s=W)

            def dv(n, r0=0, r1=H):
                return ZD[n][0:64, 4096 + r0 * W:4096 + r1 * W].rearrange(
                    "p (r s) -> p r s", r=r1 - r0, s=W)

            def interior(tns, p0, p1, r0=0, r1=H):
                return img3(tns, p0, p1)[:, 1 + r0:1 + r1, 1:1 + W]

            def rsw(r0, r1):
                o = BASEO + (1 + r0) * PW
                return slice(o, o + (r1 - r0) * PW)

            def chunk_quads(tags, body, drain):
                for g in range(2):
                    qs = [qtile(t) for t in tags]
                    for k in range(9):
                        for i in range(4):
                            body(qs[i], 4 * g + i, k)
                    for i in range(4):
                        drain(qs[i], 4 * g + i)

            QG = ("qg0", "qg1", "qg2", "qg3")
            QC = ("qc0", "qc1", "qc2", "qc3")

            # ---------- gates stream: fp8 DoubleRow span conv
            def gates_stream(n):
                Wt8 = W8[f"wg8_{n}"]

                def body(q, ci, k):
                    r0, nr = GSPANS[ci]
                    span = nr * PW
                    base = BASEO + (1 + r0) * PW + 1
                    if k < 4:
                        ta, tb = GPAIRS[k]
                        nc.tensor.matmul(
                            q[0:128, 0:span],
                            Wt8[0:96, 256 * k:256 * k + 256].rearrange(
                                "k (j m) -> k j m", j=2),
                            bass.AP(S8[n].tensor, base + ta,
                                    [[FREE, 96], [tb - ta, 2], [1, span]]),
                            start=(k == 0), stop=False, perf_mode=DRM)
                    else:
                        nc.tensor.matmul(
                            q[0:128, 0:span],
                            Wt8[0:96, 1024:1152],
                            bass.AP(S8[n].tensor, base + GSINGLE,
                                    [[FREE, 96], [1, span]]),
                            start=False, stop=True)

                def drain(q, ci):
                    # single 128-partition sigmoid drain: r -> RZ[0:64],
                    # z -> RZ[64:128] (one Act op instead of two); r*h on DVE
                    r0, nr = GSPANS[ci]
                    nc.scalar.activation(dstr(RZ[n], 0, 128, r0, nr),
                                         qspan(q, 0, 128, nr),
                                         AF.Sigmoid, bias=BIAS[0:128, n:n + 1],
                                         scale=1.0 / (SX * SW))
                    nc.vector.tensor_tensor(dstr(CX[n], 0, 64, r0, nr),
                                            dstr(RZ[n], 0, 64, r0, nr),
                                            dstr(S[n], 0, 64, r0, nr),
                                            ALU.mult)

                # 10 span chunks in groups of 4/4/2, stationaries reused
                # across the chunks of each group
                for g, gn in ((0, 4), (4, 4), (8, 2)):
                    qs = [qtile(QG[i]) for i in range(gn)]
                    for k in range(5):
                        for i in range(gn):
                            body(qs[i], g + i, k)
                    for i in range(gn):
                        drain(qs[i], g + i)
                # z to the dense layout upd expects (partition shift 64->0):
                # idle DMA engines, consumed ~15us later by upd
                nc.sync.dma_start(zv(n, 0, H),
                                  interior(RZ[n], 64, 128, 0, H))

            def cand_stream(n, rider):
                Wt = WT[("w4", "w5", "w6")[n]]
                K = 128 if rider else 96
                N = 96 if rider else 64
                ncols = 96 if n < 2 else 64

                def body(q, c, k):
                    nc.tensor.matmul(q[0:N, 0:512],
                                     Wt[0:K, k * ncols:k * ncols + N],
                                     mov(CX[n], 0, K, c, TAPS[k]),
                                     start=(k == 0), stop=(k == 8))

                def drain(q, c):
                    nc.scalar.activation(dv(n, RPC * c, RPC * c + RPC),
                                         q3(q, 0, 64),
                                         AF.Tanh, bias=BIAS[0:64, 3 + n:4 + n])
                    if rider:
                        # bu -> CX (cand moving) and S8 (gates moving), both
                        # on DVE; S[.][64:96] has no remaining reader
                        nc.vector.tensor_scalar_add(
                            dst(CX[n + 1], 64, 96, c), q3(q, 64, 96),
                            BIAS[64:96, 3 + n:4 + n])
                        nc.vector.tensor_scalar(
                            dst(S8[n + 1], 64, 96, c), q3(q, 64, 96),
                            SX, BIAS[64:96, 9 + n:10 + n],
                            ALU.mult, ALU.add)

                chunk_quads(QC, body, drain)

            def rider_stream(n):
                Wt = WT[("w4", "w5")[n]]

                def body(q, c, k):
                    nc.tensor.matmul(q[64:96, 0:512],
                                     Wt[96:128, k * 96 + 64:k * 96 + 96],
                                     mov(CX[n], 96, 128, c, TAPS[k]),
                                     start=(k == 0), stop=(k == 8),
                                     tile_position=(96, 64))

                def drain(q, c):
                    nc.vector.tensor_scalar_add(
                        dst(CX[n + 1], 64, 96, c), q3(q, 64, 96),
                        BIAS[64:96, 3 + n:4 + n])
                    nc.vector.tensor_scalar(
                        dst(S8[n + 1], 64, 96, c), q3(q, 64, 96),
                        SX, BIAS[64:96, 9 + n:10 + n],
                        ALU.mult, ALU.add)

                chunk_quads(QC, body, drain)

            def p12_body(q, c, k):
                nc.tensor.matmul(q[64:128, 0:512],
                                 WT["w7"][0:128, k * 64:k * 64 + 64],
                                 mov(HH, 0, 128, c, TAPS[k]),
                                 start=(k == 0), stop=(k == 8),
                                 tile_position=(0, 64))

            def p12_drain(q, c):
                nc.scalar.activation(dst(CX[0], 96, 128, c), q3(q, 64, 96),
                                     AF.Identity, bias=BIAS[64:96, 6:7])
                nc.scalar.activation(dst(CX[1], 96, 128, c), q3(q, 96, 128),
                                     AF.Identity, bias=BIAS[96:128, 6:7])

            def proj12_stream():
                chunk_quads(QG, p12_body, p12_drain)

            def cand2_proj12_fused():
                Wt = WT["w6"]
                for g in range(2):
                    q6 = [qtile(t) for t in QC]
                    q7 = [qtile(t) for t in QG]
                    for k in range(9):
                        for i in range(4):
                            c = 4 * g + i
                            nc.tensor.matmul(q6[i][0:64, 0:512],
                                             Wt[0:96, k * 64:k * 64 + 64],
                                             mov(CX[2], 0, 96, c, TAPS[k]),
                                             start=(k == 0), stop=(k == 8))
                            p12_body(q7[i], c, k)
                    for i in range(4):
                        c = 4 * g + i
                        nc.scalar.activation(dv(2, RPC * c, RPC * c + RPC),
                                             q3(q6[i], 0, 64), AF.Tanh,
                                             bias=BIAS[0:64, 5:6])
                        p12_drain(q7[i], c)

            def proj0_stream(g):
                nt = 2 if g == 2 else 4
                pb, K, N = 32 * g, 3 * nt, 32 * nt

                def body(q, c, k):
                    nc.tensor.matmul(q[0:N, 0:512],
                                     WT["wp0"][pb:pb + K, k * 128:k * 128 + N],
                                     mov(XA, pb, pb + K, c, TAPS[k]),
                                     start=(k == 0), stop=(k == 8))

                def drain(q, c):
                    nc.scalar.activation(dst(PA, 0, N, c, img=g), q3(q, 0, N),
                                         AF.Identity, bias=BIAS[0:N, 7:8])

                chunk_quads(QG, body, drain)

            def integ0_stream(g):
                nt = 2 if g == 2 else 4
                K = N = 32 * nt

                def body(q, c, k):
                    nc.tensor.matmul(q[0:N, 0:512],
                                     WT["wi0"][0:K, k * 128:k * 128 + N],
                                     mov(PA, 0, K, c, TAPS[k], img=g),
                                     start=(k == 0), stop=(k == 8))

                def drain(q, c):
                    nc.scalar.activation(dst(BA, 0, N, c, img=g), q3(q, 0, N),
                                         AF.Identity, bias=BIAS[0:N, 8:9])
                    nc.scalar.activation(dst(BA8, 0, N, c, img=g), q3(q, 0, N),
                                         AF.Identity,
                                         bias=BIAS[0:N, 11:12], scale=SX)

                chunk_quads(QC, body, drain)

            def copy_bu0(t):
                g, u = t // 4, t % 4
                nc.sync.dma_start(CX[0][64:96, sw],
                                  BA[32 * u:32 * u + 32, swi(g)])
                nc.sync.dma_start(S8[0][64:96, sw],
                                  BA8[32 * u:32 * u + 32, swi(g)])

            def upd(n, eng=None, r0=0, r1=H):
                # h' = d + z*(h - d); CX[n] interior used as scratch.
                eng = eng or nc.vector
                ci = interior(CX[n], 0, 64, r0, r1)
                si = interior(S[n], 0, 64, r0, r1)
                eng.tensor_tensor(ci, si, dv(n, r0, r1), ALU.subtract)
                eng.tensor_tensor(ci, zv(n, r0, r1), ci, ALU.mult)
                eng.tensor_tensor(si, dv(n, r0, r1), ci, ALU.add)

            def s8h(n, r0=0, r1=H):
                # refresh the fp8 gates moving copy S8[n] = h * SX (DVE)
                nc.vector.tensor_scalar_mul(
                    interior(S8[n], 0, 64, r0, r1),
                    interior(S[n], 0, 64, r0, r1), SX)

            # ---------- program
            for t in range(T):
                pb = 32 * (t // 4) + 3 * (t % 4)
                nc.sync.dma_start(img3(XA, pb, pb + 3)[:, 1:1 + H, 1:1 + W],
                                  x_ext[t])

            for rep in range(n_repeat):
                for n in range(3):
                    nc.gpsimd.memset(S[n][0:64, :], 0.0)
                    nc.gpsimd.memset(S8[n][0:64, :], 0.0)
                for g in range(2):
                    proj0_stream(g)
                for g in range(2):
                    integ0_stream(g)

                for t in range(PROCESS_T):
                    if t == 0:
                        copy_bu0(0)
                        gates_stream(0)                      # G0(0)
                    if t <= 7:
                        cand_stream(0, rider=(t >= 1))       # C0(t)
                    elif t == 8:
                        rider_stream(0)                      # bu1[8] only
                    if t <= 6:
                        copy_bu0(t + 1)      # early: hoisted G0(t+1) needs it
                    if t <= 7:
                        for hf in (0, 1):                    # upd0 + h0->HH
                            upd(0, r0=32 * hf, r1=32 * hf + 32)
                            nc.sync.dma_start(
                                HH[64:128, rsw(32 * hf, 32 * hf + 32)],
                                S[0][0:64, rsw(32 * hf, 32 * hf + 32)])
                            if t <= 6:       # h0[7] never feeds gates0 again
                                s8h(0, 32 * hf, 32 * hf + 32)
                    if t == 0:
                        nc.sync.dma_start(HH[0:64, sw], S[1][0:64, sw])
                    if 1 <= t <= 8:
                        gates_stream(1)                      # G1(t)
                        cand_stream(1, rider=True)           # C1(t)
                    elif t == 9:
                        rider_stream(1)                      # bu2[9]
                    if 1 <= t <= 8:
                        for hf in (0, 1):                    # upd1 + h1->HH
                            upd(1, r0=32 * hf, r1=32 * hf + 32)
                            nc.sync.dma_start(
                                HH[0:64, rsw(32 * hf, 32 * hf + 32)],
                                S[1][0:64, rsw(32 * hf, 32 * hf + 32)])
                            if t <= 7:       # h1[8] never feeds gates1 again
                                s8h(1, 32 * hf, 32 * hf + 32)
                    if t >= 2:
                        gates_stream(2)                      # G2(t)
                    if t <= 6:
                        gates_stream(0)      # G0(t+1) hoisted: fills the PE
                                             # while cand2's inputs settle
                    if 2 <= t <= 8:
                        cand2_proj12_fused()                 # C2(t) || p12
                    elif t == 9:
                        cand_stream(2, rider=False)          # C2(9)
                    elif t <= 1:
                        proj12_stream()      # p1/p2 startup (t+1)
                    if t >= 2:
                        # t=8 is the tail-critical step: gates2(9) waits on
                        # h2[8]; Pool's 0.42-efficiency chain would stall PE
                        upd(2, nc.vector if t == 8 else nc.gpsimd)
                        if t <= 8:           # h2[9] is the output only
                            s8h(2)

                # output h2 (f32)
                hv = img3(S[2], 0, 64)
                nc.vector.tensor_copy(
                    OUTF[0:64, :].rearrange("p (r s) -> p r s", r=H // 2, s=W),
                    hv[:, 1:1 + H // 2, 1:1 + W])
                nc.scalar.activation(
                    OUTF[64:128, :].rearrange("p (r s) -> p r s", r=H // 2,
                                              s=W),
                    hv[:, 1 + H // 2:1 + H, 1:1 + W], AF.Identity)
                nc.sync.dma_start(
                    out_ext[:, 0:H // 2, :],
                    OUTF[0:64, :].rearrange("p (r s) -> p r s", r=H // 2, s=W))
                nc.sync.dma_start(
                    out_ext[:, H // 2:H, :],
                    OUTF[64:128, :].rearrange("p (r s) -> p r s", r=H // 2,
                                              s=W))

    nc.compile()
    return nc


# ----------------------------------------------------------------- entry
def kernel(**inputs) -> np.ndarray:
    from concourse.bass_utils import run_bass_kernel_spmd
    xb, w, bias = _prep_inputs(inputs)
    if "nc" not in _cache:
        _cache["nc"] = build(1)
    nc = _cache["nc"]
    in_maps = []
    for b in range(N_CORES):
        m = {"x": np.ascontiguousarray(xb[b]), "bias": bias}
        m.update(w)
        in_maps.append(m)
    core_ids = list(range(N_CORES))
    try:
        res = run_bass_kernel_spmd(nc, in_maps, core_ids=core_ids).results
    except Exception:
        # transient device wedge has been observed; a clean retry recovers it
        res = run_bass_kernel_spmd(nc, in_maps, core_ids=core_ids).results
    return np.stack([res[b]["out"] for b in range(N_CORES)]).astype(np.float32)
